# revision 28
# baseline (speedup 1.0000x reference)
"""Trainium2 Bass kernel for top-2-of-8 MoE (T=4096, H=1024, I=1024).

Strategy (sparse routed grouped-GEMM, expert-sharded, 8 cores):
  - Routing (softmax + top-2 + renormalize) is computed on the HOST from the
    router logits (T x 8 — trivial), giving per-pair (token, expert, weight).
  - Each core owns exactly ONE expert: its full up/down weights (6 MB bf16)
    plus only the tokens routed to it (~1024 of 8192 pairs), padded to a
    compile-time capacity C (multiple of 128).
  - Device dataflow is transpose-free:
      up:   hT[i_chunk, pairs] = Wup[h, i_chunk].T @ xT[h, pairs]
            (weights stationary, token columns streamed; output is h
             TRANSPOSED with I on partitions — exactly what down needs)
      act:  h = silu(gate) * up     (ACT Silu + DVE multiply)
      down: y[pair_tile, H] = hT[:, pair_tile].T @ Wdn[i, H]
            (PSUM-accumulated over the 8 I-chunks)
      scale: y *= combine_weight (per-partition scalar on ACT) -> DMA out bf16
  - No collectives: each pair's full down-projection lives on one core.
    The host gathers per-core pair rows and adds the two pairs per token.

Compute dtype bf16 (f32 PSUM accumulation), bf16 device output upcast on host.
"""

import os
import sys

for _p in ("/opt/trn_rl_repo",):
    if _p not in sys.path:
        sys.path.append(_p)

import numpy as np
import ml_dtypes

import concourse.bass as bass
import concourse.bacc as bacc
import concourse.mybir as mybir
import concourse.tile as tile
from concourse.bass_utils import run_bass_kernel_spmd

BF16 = mybir.dt.bfloat16
F32 = mybir.dt.float32
AX = mybir.AxisListType
OP = mybir.AluOpType
AF = mybir.ActivationFunctionType

N_CORES = 8
H = 1024
I_FULL = 1024
E = 8
K_TOP = 2
KT = H // 128  # 8 contraction k-tiles for the up GEMM
IC = I_FULL // 128  # 8 I-chunks
P = 128


def _rearrange(x, pattern, **kw):
    import einops

    return np.ascontiguousarray(einops.rearrange(x, pattern, **kw))


def _chunks(C):
    out = []
    c0 = 0
    while c0 < C:
        cw = min(512, C - c0)
        out.append((c0, cw))
        c0 += cw
    return out


def build_graph(C):
    """SPMD graph: one expert per core, capacity C pairs (multiple of 128)."""
    NTI = C // P  # pair tiles
    chunks = _chunks(C)

    nc = bacc.Bacc("TRN2", target_bir_lowering=False, debug=False,
                   num_devices=N_CORES)

    xt_ext = nc.dram_tensor("xt", [P, KT * C], BF16, kind="ExternalInput")
    wup_ext = nc.dram_tensor("wup", [P, IC * 2048], BF16, kind="ExternalInput")
    wd_ext = nc.dram_tensor("wd", [P, IC * H], BF16, kind="ExternalInput")
    wsc_ext = nc.dram_tensor("wsc", [P, NTI], F32, kind="ExternalInput")
    out_ext = nc.dram_tensor("out", [C, H], BF16, kind="ExternalOutput")

    with tile.TileContext(nc) as tc:
        with (
            tc.tile_pool(name="big", bufs=1) as big,
            tc.tile_pool(name="work", bufs=2) as work,
            tc.tile_pool(name="hbuf", bufs=1) as hbuf,
            tc.tile_pool(name="outp", bufs=2) as outp,
            tc.tile_pool(name="pup", bufs=1, space="PSUM") as pup,
            tc.tile_pool(name="pdn", bufs=1, space="PSUM") as pdn,
        ):
            xt = big.tile([P, KT * C], BF16)
            wup = big.tile([P, IC * 2048], BF16)
            wd = big.tile([P, IC * H], BF16)
            wsc = big.tile([P, NTI], F32)

            # Warmup: junk matmuls (never-written SBUF, discarded PSUM) fill
            # the PE's DMA-wait window at NEFF start and ramp its p-state to
            # max before the first real matmul. Same-bank, same-engine, so
            # they add no semaphores and the first real pg write just queues
            # behind them in PE program order.
            warm_l = big.tile([P, P], BF16)
            warm_r = big.tile([P, 512], BF16)
            nc.gpsimd.memset(warm_l[:], 0.0)
            nc.gpsimd.memset(warm_r[:], 0.0)
            for w in range(6):
                pwm = pup.tile([P, 512], F32, tag="pg0", name="warm%d" % w)
                nc.tensor.matmul(pwm[:], warm_l[:], warm_r[:],
                                 start=True, stop=True)

            # All DMAs on the sync queue; transfers complete roughly FIFO at
            # ~356 GB/s aggregate, so the stream is ordered by first PE use:
            # chunk-0 tokens + wup0 (the first matmul's needs) lead.
            c0, cw = chunks[0]
            nc.sync.dma_start(xt[:, 0 * C + c0: 0 * C + c0 + cw],
                              xt_ext[:, 0 * C + c0: 0 * C + c0 + cw])
            nc.sync.dma_start(wup[:, 0:2048], wup_ext[:, 0:2048])
            for k in range(1, KT):
                nc.sync.dma_start(xt[:, k * C + c0: k * C + c0 + cw],
                                  xt_ext[:, k * C + c0: k * C + c0 + cw])
            for ip in range(1, IC):
                nc.sync.dma_start(wup[:, ip * 2048:(ip + 1) * 2048],
                                  wup_ext[:, ip * 2048:(ip + 1) * 2048])
            for ip in range(2):
                nc.sync.dma_start(wd[:, ip * H:(ip + 1) * H],
                                  wd_ext[:, ip * H:(ip + 1) * H])
            if len(chunks) > 1:
                c0, cw = chunks[1]
                for k in range(KT):
                    nc.sync.dma_start(xt[:, k * C + c0: k * C + c0 + cw],
                                      xt_ext[:, k * C + c0: k * C + c0 + cw])
            for ip in range(2, IC):
                nc.sync.dma_start(wd[:, ip * H:(ip + 1) * H],
                                  wd_ext[:, ip * H:(ip + 1) * H])
            nc.sync.dma_start(wsc[:], wsc_ext[:])
            for (c0, cw) in chunks[2:]:
                for k in range(KT):
                    nc.sync.dma_start(xt[:, k * C + c0: k * C + c0 + cw],
                                      xt_ext[:, k * C + c0: k * C + c0 + cw])

            hT = {}

            def up_chunk(cc):
                c0, cw = chunks[cc]
                gen = cc % 2
                for ip in range(IC):
                    pg = pup.tile([P, 512], F32, tag="pg%d" % (ip % 2),
                                  name="pg_%d_%d" % (cc, ip))[:]
                    pu = pup.tile([P, 512], F32, tag="pu%d" % (ip % 2),
                                  name="pu_%d_%d" % (cc, ip))[:]
                    for k in range(KT):
                        w0 = ip * 2048 + k * 256
                        nc.tensor.matmul(
                            pg[:, :cw], wup[:, w0: w0 + 128],
                            xt[:, k * C + c0: k * C + c0 + cw],
                            start=(k == 0), stop=(k == KT - 1))
                    for k in range(KT):
                        w0 = ip * 2048 + k * 256 + 128
                        nc.tensor.matmul(
                            pu[:, :cw], wup[:, w0: w0 + 128],
                            xt[:, k * C + c0: k * C + c0 + cw],
                            start=(k == 0), stop=(k == KT - 1))
                    sg = work.tile([P, 512], F32, tag="sg")
                    nc.scalar.activation(sg[:, :cw], pg[:, :cw], AF.Silu)
                    ht = hbuf.tile([P, 512], BF16, tag="h%d_%d" % (gen, ip),
                                   name="h_%d_%d" % (cc, ip))
                    nc.vector.tensor_tensor(ht[:, :cw], sg[:, :cw],
                                            pu[:, :cw], op=OP.mult)
                    hT[(gen, ip)] = ht

            def down_chunk(cc):
                c0, cw = chunks[cc]
                gen = cc % 2
                for tt in range(cw // P):
                    gt = c0 // P + tt
                    y0 = pdn.tile([P, 512], F32, tag="y0%d" % (tt % 2),
                                  name="y0_%d" % gt)
                    y1 = pdn.tile([P, 512], F32, tag="y1%d" % (tt % 2),
                                  name="y1_%d" % gt)
                    for ip in range(IC):
                        lhs = hT[(gen, ip)][:, tt * P: (tt + 1) * P]
                        nc.tensor.matmul(y0[:], lhs,
                                         wd[:, ip * H: ip * H + 512],
                                         start=(ip == 0), stop=(ip == IC - 1))
                        nc.tensor.matmul(y1[:], lhs,
                                         wd[:, ip * H + 512: (ip + 1) * H],
                                         start=(ip == 0), stop=(ip == IC - 1))
                    # scale+store per half: y0's half ships while y1's last
                    # matmul and scale still run, shortening the end chain.
                    ysb = outp.tile([P, H], BF16, tag="ysb")
                    nc.scalar.mul(ysb[:, 0:512], y0[:], wsc[:, gt: gt + 1])
                    nc.sync.dma_start(out_ext[gt * P:(gt + 1) * P, 0:512],
                                      ysb[:, 0:512])
                    nc.scalar.mul(ysb[:, 512:H], y1[:], wsc[:, gt: gt + 1])
                    nc.sync.dma_start(out_ext[gt * P:(gt + 1) * P, 512:H],
                                      ysb[:, 512:H])

            # software pipeline: down(cc-1) is emitted after up(cc) so the PE
            # queue never stalls waiting for the activation of chunk cc.
            for cc in range(len(chunks)):
                up_chunk(cc)
                if cc > 0:
                    down_chunk(cc - 1)
            down_chunk(len(chunks) - 1)

    nc.compile()
    return nc


def route(router_logits):
    """Host top-2 routing, bit-matching the reference's top_k semantics."""
    T = router_logits.shape[0]
    m = router_logits.max(-1, keepdims=True)
    ex = np.exp(router_logits - m)
    p = ex / ex.sum(-1, keepdims=True)
    rows = np.arange(T)
    a1 = np.argmax(p, axis=-1)
    p1 = p[rows, a1]
    pm = p.copy()
    pm[rows, a1] = -1.0
    a2 = np.argmax(pm, axis=-1)
    p2 = p[rows, a2]
    s = p1 + p2
    return a1, a2, p1 / s, p2 / s


def make_in_maps(hidden_states, router_logits, up_weight, down_weight):
    """Host routing + per-core (per-expert) input prep.

    Returns (in_maps, pos, C): pos[t, slot] is the row in the concatenated
    [8*C, H] device output holding that pair's (already weighted) result.
    """
    T = hidden_states.shape[0]
    bf = ml_dtypes.bfloat16
    a1, a2, w1, w2 = route(router_logits.astype(np.float32))
    counts = np.bincount(a1, minlength=E) + np.bincount(a2, minlength=E)
    C = max(1152, int(-(-counts.max() // P) * P))

    x16 = hidden_states.astype(bf)
    pos = np.empty((T, 2), dtype=np.int64)
    in_maps = []
    for e in range(E):
        t1 = np.flatnonzero(a1 == e)
        t2 = np.flatnonzero(a2 == e)
        pos[t1, 0] = e * C + np.arange(len(t1))
        pos[t2, 1] = e * C + len(t1) + np.arange(len(t2))
        cnt = len(t1) + len(t2)

        xpad = np.zeros((C, H), dtype=bf)
        xpad[:len(t1)] = x16[t1]
        xpad[len(t1):cnt] = x16[t2]
        xt = _rearrange(xpad, "c (k p) -> p (k c)", p=P)

        wpad = np.zeros((C,), dtype=np.float32)
        wpad[:len(t1)] = w1[t1]
        wpad[len(t1):cnt] = w2[t2]
        wsc = _rearrange(wpad, "(t p) -> p t", p=P)

        W = up_weight[e].astype(bf)
        Wg = W[:, :I_FULL].reshape(KT, P, IC, P)
        Wu = W[:, I_FULL:].reshape(KT, P, IC, P)
        wup = _rearrange(np.stack([Wg, Wu], axis=3), "k p i s q -> p (i k s q)")

        wdn = _rearrange(down_weight[e].astype(bf), "(i p) h -> p (i h)", p=P)

        in_maps.append({"xt": xt, "wup": wup, "wd": wdn, "wsc": wsc})
    return in_maps, pos, C


_GRAPH_CACHE = {}


def _get_graph(C):
    if C not in _GRAPH_CACHE:
        _GRAPH_CACHE[C] = build_graph(C)
    return _GRAPH_CACHE[C]


def kernel(hidden_states, router_logits, up_weight, down_weight, topk,
           trace=False):
    assert int(topk) == K_TOP
    hidden_states = np.asarray(hidden_states, dtype=np.float32)
    router_logits = np.asarray(router_logits, dtype=np.float32)
    up_weight = np.asarray(up_weight, dtype=np.float32)
    down_weight = np.asarray(down_weight, dtype=np.float32)

    in_maps, pos, C = make_in_maps(hidden_states, router_logits,
                                   up_weight, down_weight)
    nc = _get_graph(C)
    res = run_bass_kernel_spmd(nc, in_maps, list(range(N_CORES)), trace=trace)
    Y = np.concatenate([res.results[r]["out"].astype(np.float32)
                        for r in range(N_CORES)], axis=0)
    out = Y[pos[:, 0]] + Y[pos[:, 1]]
    kernel.last_exec_time_ns = res.exec_time_ns
    return out


kernel.last_exec_time_ns = None
